# revision 29
# baseline (speedup 1.0000x reference)
"""Trainium2 Bass kernel for a 6-layer transformer encoder (B=4, S=1024,
d_model=1024, 16 heads, d_ff=4096).

Sharding: token-parallel across 8 cores (B*S = 4096 tokens -> 512/core; each
core owns half of one batch element's sequence).  Per layer, one pair-wise
AllGather of the bf16 K^T / V activations lets each core rebuild K/V for its
full batch element; Q/attention-rows/O-proj/FFN/LayerNorms are computed only
for the core's own 512 tokens.

On-chip layout: activations are kept transposed (d_model on partitions,
tokens on free dim).  Matmuls run in bf16 with fp32 PSUM accumulation; the
residual stream and LayerNorm math stay fp32.

v2 restructure vs baseline:
- the attention mask is folded into V and into the per-head mask-ones columns
  of the augmented V (zeroing masked keys in numerator and denominator), so
  exp() takes no bias and batches over two PSUM banks per instruction.
- each head's augmented V is [V | mask01] (even heads) / [mask01 | V] (odd
  heads): the attnV matmul also produces 64 partition-aligned copies of the
  softmax denominator; one fast-approx reciprocal per head half, one
  SBUF->SBUF DMA swapping halves, and one DVE multiply write the normalized
  bf16 head rows straight into the attn tile (no DRAM bounce, no 1-row ops).
- scores matmuls of a head pair interleave the two 64-row PE tiles so
  LDWEIGHTS overlaps the other half's matmul; attnV of pair j issues after
  the scores of pair j+1 (software pipeline keeping Act busy).
- LayerNorm rstd = exp(-0.5*ln(var+eps)): Scalar engine stays on the
  natural_log_exp activation-table set for the whole kernel.
- PSUM statically tiled: 2x [128,2,512] chunk slots + 4x [128,512] banks.
"""

import sys
import os

for _p in ("/opt/trn_rl_repo", "/root/.axon_site/_ro/trn_rl_repo"):
    if os.path.isdir(_p) and _p not in sys.path:
        sys.path.insert(0, _p)

import numpy as np
import ml_dtypes

import concourse.bass as bass
import concourse.mybir as mybir
import concourse.tile as tile
from concourse.bass_utils import run_bass_kernel_spmd
from concourse.masks import make_identity

VOCAB, D, H, DFF, L = 32000, 1024, 16, 4096, 6
B, S = 4, 1024
DK = D // H              # 64
NCORES = 8
TOK = (B * S) // NCORES  # 512 tokens per core
KT = D // 128            # 8
FT = DFF // 128          # 32
EPS = 1e-5

F32 = mybir.dt.float32
BF16 = mybir.dt.bfloat16
I32 = mybir.dt.int32
AF = mybir.ActivationFunctionType
OP = mybir.AluOpType

_NC = None


def _build_nc():
    nc = bass.Bass("TRN2", target_bir_lowering=False, debug=False, num_devices=NCORES)

    emb = nc.dram_tensor("emb", [VOCAB, D], F32, kind="ExternalInput")
    src = nc.dram_tensor("src", [TOK, 1], I32, kind="ExternalInput")
    peT = nc.dram_tensor("peT", [D, TOK], F32, kind="ExternalInput")
    mask01d = nc.dram_tensor("mask01", [128, KT], F32, kind="ExternalInput")
    maskrowd = nc.dram_tensor("maskrow", [128, KT, 64], BF16, kind="ExternalInput")
    koidx = nc.dram_tensor("koidx", [TOK, 1], I32, kind="ExternalInput")
    voidx = nc.dram_tensor("voidx", [TOK, 1], I32, kind="ExternalInput")
    wq = nc.dram_tensor("wq", [L, D, D], BF16, kind="ExternalInput")
    wk = nc.dram_tensor("wk", [L, D, D], BF16, kind="ExternalInput")
    wv = nc.dram_tensor("wv", [L, D, D], BF16, kind="ExternalInput")
    wo = nc.dram_tensor("wo", [L, D, D], BF16, kind="ExternalInput")
    w1 = nc.dram_tensor("w1", [L, D, DFF], BF16, kind="ExternalInput")
    w2 = nc.dram_tensor("w2", [L, DFF, D], BF16, kind="ExternalInput")
    bqT = nc.dram_tensor("bqT", [L, 128, KT], F32, kind="ExternalInput")
    bkT = nc.dram_tensor("bkT", [L, 128, KT], F32, kind="ExternalInput")
    boT = nc.dram_tensor("boT", [L, 128, KT], F32, kind="ExternalInput")
    b1T = nc.dram_tensor("b1T", [L, 128, FT], F32, kind="ExternalInput")
    b2T = nc.dram_tensor("b2T", [L, 128, KT], F32, kind="ExternalInput")
    g1T = nc.dram_tensor("g1T", [L, 128, KT], F32, kind="ExternalInput")
    be1T = nc.dram_tensor("be1T", [L, 128, KT], F32, kind="ExternalInput")
    g2T = nc.dram_tensor("g2T", [L, 128, KT], F32, kind="ExternalInput")
    be2T = nc.dram_tensor("be2T", [L, 128, KT], F32, kind="ExternalInput")
    xout = nc.dram_tensor("xout", [D, TOK], F32, kind="ExternalOutput")

    with tile.TileContext(nc) as tc:
        with (
            tc.tile_pool(name="cpool", bufs=1) as cpool,
            tc.tile_pool(name="wp", bufs=3) as wp,
            tc.tile_pool(name="w2p", bufs=4) as w2p,
            tc.tile_pool(name="p1", bufs=1) as p1,
            tc.tile_pool(name="p2", bufs=2) as p2,
            tc.tile_pool(name="bp", bufs=8) as bp,
            tc.tile_pool(name="ps", bufs=1, space="PSUM") as ps,
            tc.tile_pool(name="dram", bufs=2, space="DRAM") as dram,
        ):
            _uid = [0]

            def _nm(tag):
                _uid[0] += 1
                return f"{tag}_{_uid[0]}"

            ident = cpool.tile([128, 128], BF16, tag="ident", name=_nm("ident"))
            make_identity(nc, ident[:])
            onesk = cpool.tile([128, 128], BF16, tag="onesk", name=_nm("onesk"))
            nc.vector.memset(onesk[:], 1.0 / D)
            eps_sb = cpool.tile([128, 1], F32, tag="eps", name=_nm("eps"))
            nc.vector.memset(eps_sb[:], EPS)
            vaug = cpool.tile([128, KT, H, 128], BF16, tag="vaug", name=_nm("vaug"))

            # PSUM static plan: 2 chunk slots of [128,3,512] + 2 of [128,512]
            def psA():
                return ps.tile([128, 3, 512], F32, tag="psA", bufs=2, name=_nm("psA"))

            def psB():
                return ps.tile([128, 512], F32, tag="psB", bufs=2, name=_nm("psB"))

            def dense_groups():
                """4 independent accumulation banks: one psA (3 banks) + one
                psB."""
                a = psA()
                b = psB()
                return [a[:, 0, :], a[:, 1, :], a[:, 2, :], b]

            def load_bias8(t, l):
                b = bp.tile([128, KT], F32, tag="bias8", name=_nm("bias8"))
                nc.sync.dma_start(b[:], t[l])
                return b

            # ---------------- embedding ----------------
            # issue the index DMAs + gathers before the constant-table DMAs so
            # the PE-feeding chain starts immediately.
            gats = []
            for blk in range(TOK // 128):
                idx_t = p2.tile([128, 1], I32, tag="idx", name=_nm("idx"))
                nc.sync.dma_start(idx_t[:], src[blk * 128:(blk + 1) * 128, :])
                gat = p1.tile([128, D], F32, tag=("attn" if blk % 2 else "kto"),
                              name=_nm("gat"))
                nc.gpsimd.indirect_dma_start(
                    out=gat[:], out_offset=None, in_=emb[:],
                    in_offset=bass.IndirectOffsetOnAxis(ap=idx_t[:, :1], axis=0),
                )
                gats.append(gat)
            peT_sb = p1.tile([128, KT, TOK], F32, tag="big", name=_nm("peT"))
            nc.sync.dma_start(peT_sb[:], peT.rearrange("(t p) n -> p t n", p=128))
            mask01 = cpool.tile([128, KT], F32, tag="mask01", name=_nm("mask01"))
            nc.sync.dma_start(mask01[:], mask01d[:])
            # augmented V: per head 128 cols; even head h: [V_h | mask01],
            # odd head h: [mask01 | V_h].  The mask01 blocks are written once
            # by broadcast DMA (mask constant across layers); V blocks are
            # rewritten per layer.
            for kt in range(KT):
                nc.sync.dma_start(
                    vaug[:, kt, 0:H:2, 64:128],
                    maskrowd[:, kt, None, :].to_broadcast((128, 8, 64)))
                nc.sync.dma_start(
                    vaug[:, kt, 1:H:2, 0:64],
                    maskrowd[:, kt, None, :].to_broadcast((128, 8, 64)))
            x_cur = p2.tile([128, KT, TOK], F32, tag="x", bufs=1, name=_nm("x"))
            for blk in range(TOK // 128):
                gat = gats[blk]
                gatb = p2.tile([128, D], BF16, tag="rec", bufs=1, name=_nm("gatb"))
                nc.scalar.activation(gatb[:], gat[:], AF.Copy)
                for kt in range(KT):
                    tp = ps.tile([128, 128], BF16, tag="psB", bufs=2, name=_nm("tp"))
                    nc.tensor.transpose(tp[:], gatb[:, kt * 128:(kt + 1) * 128],
                                        ident[:])
                    nc.vector.scalar_tensor_tensor(
                        out=x_cur[:, kt, blk * 128:(blk + 1) * 128],
                        in0=tp[:], scalar=32.0,
                        in1=peT_sb[:, kt, blk * 128:(blk + 1) * 128],
                        op0=OP.mult, op1=OP.add,
                    )

            # ---------------- helpers ----------------
            def layer_norm(r, g_sb, be_sb, want_f32=False):
                """r: [128, KT, TOK] f32 residual -> xb bf16 (or xo f32)."""
                # rb/sq reuse the exps slots (dead between attention phases)
                rb = p2.tile([128, KT, TOK], BF16, tag="exps", name=_nm("rb"))
                sq = p2.tile([128, KT, TOK], BF16, tag="exps", name=_nm("sq"))
                for kk in range(KT):
                    if kk % 2 == 0:
                        nc.scalar.activation(rb[:, kk, :], r[:, kk, :], AF.Copy)
                    else:
                        nc.vector.tensor_copy(rb[:, kk, :], r[:, kk, :])
                for kk in range(KT):
                    nc.vector.tensor_mul(sq[:, kk, :], rb[:, kk, :], rb[:, kk, :])
                pmu = psB()
                pm2 = psB()
                for kk in range(KT):
                    nc.tensor.matmul(pmu[:], onesk[:], rb[:, kk, :],
                                     start=(kk == 0), stop=(kk == KT - 1))
                for kk in range(KT):
                    nc.tensor.matmul(pm2[:], onesk[:], sq[:, kk, :],
                                     start=(kk == 0), stop=(kk == KT - 1))
                musq = p2.tile([128, TOK], F32, tag="lns", bufs=5, name=_nm("musq"))
                nc.scalar.activation(musq[:], pmu[:], AF.Square)
                var = p2.tile([128, TOK], F32, tag="lns", bufs=5, name=_nm("var"))
                nc.vector.tensor_sub(var[:], pm2[:], musq[:])
                lnv = p2.tile([128, TOK], F32, tag="lns", bufs=5, name=_nm("lnv"))
                nc.scalar.activation(lnv[:], var[:], AF.Ln, bias=eps_sb[:, 0:1])
                rstd = p2.tile([128, TOK], F32, tag="lns", bufs=5, name=_nm("rstd"))
                nc.scalar.activation(rstd[:], lnv[:], AF.Exp, scale=-0.5)
                # z = mu * rstd  (both broadcast over partitions already)
                z = p2.tile([128, TOK], F32, tag="lns", bufs=5, name=_nm("z"))
                nc.vector.tensor_mul(z[:], pmu[:], rstd[:])
                if want_f32:
                    xb = p1.tile([128, KT, TOK], F32, tag="big", name=_nm("xof"))
                else:
                    xb = p2.tile([128, KT, TOK], BF16, tag="xcb", name=_nm("xb"))
                for kk in range(KT):
                    # z2 = (z * g) - be ; t = (r * g) * rstd ; y = t - z2
                    z2 = p2.tile([128, TOK], F32, tag="lns2", name=_nm("z2"))
                    nc.vector.tensor_scalar(
                        z2[:], z[:], g_sb[:, kk:kk + 1], be_sb[:, kk:kk + 1],
                        OP.mult, OP.subtract)
                    t = p2.tile([128, TOK], F32, tag="lns2", name=_nm("t"))
                    nc.vector.scalar_tensor_tensor(
                        out=t[:], in0=r[:, kk, :], scalar=g_sb[:, kk:kk + 1],
                        in1=rstd[:], op0=OP.mult, op1=OP.mult)
                    nc.vector.tensor_sub(xb[:, kk, :], t[:], z2[:])
                return xb

            # ---------------- layers ----------------
            x_curb = p2.tile([128, KT, TOK], BF16, tag="xcb", name=_nm("xcb0"))
            for kk in range(KT):
                nc.scalar.activation(x_curb[:, kk, :], x_cur[:, kk, :], AF.Copy)

            xo_final = None
            for l in range(L):
                bq_sb = load_bias8(bqT, l)
                bk_sb = load_bias8(bkT, l)

                # --- K/V/Q projections, halves round-robined so PSUM
                # evacuations and the K/V AllGathers overlap the next MM
                # group.  K ships per feature-half (2 small collectives);
                # V ships per head-parity (even/odd collectives) so the
                # pair gathers can write straight into vaug.
                ktl = p1.tile([128, KT, TOK], BF16, tag="ktl", name=_nm("ktl"))
                qt = p1.tile([128, KT, TOK], BF16, tag="qt", name=_nm("qt"))
                kag_in = [dram.tile([512, TOK], BF16, tag=f"kag_in{h}", bufs=2,
                                    name=_nm("kag_in")) for h in range(2)]
                kag_out = [dram.tile([2 * 512, TOK], BF16, tag=f"kag_out{h}",
                                     bufs=2, name=_nm("kag_out"))
                           for h in range(2)]
                vagE_in = dram.tile([TOK, 512], BF16, tag="vagE_in", bufs=2,
                                    name=_nm("vagE_in"))
                vagO_in = dram.tile([TOK, 512], BF16, tag="vagO_in", bufs=2,
                                    name=_nm("vagO_in"))
                vagE_out = dram.tile([2 * TOK, 512], BF16, tag="vagE_out",
                                     bufs=2, name=_nm("vagE_out"))
                vagO_out = dram.tile([2 * TOK, 512], BF16, tag="vagO_out",
                                     bufs=2, name=_nm("vagO_out"))
                pair_groups = [[2 * i, 2 * i + 1] for i in range(NCORES // 2)]
                def k_half(half):
                    wkh = wp.tile([128, KT, 512], BF16, tag="wproj", name=_nm("wk"))
                    nc.sync.dma_start(
                        wkh[:], wk[l, :, half * 512:(half + 1) * 512]
                        .rearrange("(t p) m -> p t m", p=128))
                    grps = dense_groups()
                    for m in range(4):
                        mg = half * 4 + m
                        pt = grps[m]
                        for kk in range(KT):
                            nc.tensor.matmul(
                                pt[:], wkh[:, kk, m * 128:(m + 1) * 128],
                                x_curb[:, kk, :],
                                start=(kk == 0), stop=(kk == KT - 1))
                        nc.scalar.activation(ktl[:, mg, :], pt[:], AF.Identity,
                                             bias=bk_sb[:, mg:mg + 1])
                        nc.sync.dma_start(
                            kag_in[half][m * 128:(m + 1) * 128, :],
                            ktl[:, mg, :])
                    nc.gpsimd.collective_compute(
                        "AllGather", OP.bypass,
                        ins=[kag_in[half][:]], outs=[kag_out[half][:]],
                        replica_groups=pair_groups,
                    )

                def v_half(half):
                    wvh = wp.tile([128, KT, 512], BF16, tag="wproj", name=_nm("wv"))
                    nc.sync.dma_start(
                        wvh[:], wv[l, :, half * 512:(half + 1) * 512]
                        .rearrange("(t p) m -> p t m", p=128))
                    hb = half * 8
                    grps = dense_groups()
                    for mt in range(4):   # own token tiles
                        pt = grps[mt]
                        for kk in range(KT):
                            nc.tensor.matmul(
                                pt[:], x_curb[:, kk, mt * 128:(mt + 1) * 128],
                                wvh[:, kk, :],
                                start=(kk == 0), stop=(kk == KT - 1))
                        ptv = pt.rearrange("p (h c) -> p h c", c=64)
                        nc.scalar.activation(
                            vaug[:, mt, hb:hb + 8:2, 0:64], ptv[:, 0:8:2, :],
                            AF.Copy, scale=mask01[:, mt:mt + 1])
                        nc.vector.tensor_scalar_mul(
                            vaug[:, mt, hb + 1:hb + 8:2, 64:128],
                            ptv[:, 1:8:2, :], mask01[:, mt:mt + 1])
                        nc.sync.dma_start(
                            vagE_in[mt * 128:(mt + 1) * 128,
                                    half * 256:(half + 1) * 256]
                            .rearrange("p (h c) -> p h c", c=64),
                            vaug[:, mt, hb:hb + 8:2, 0:64])
                        nc.sync.dma_start(
                            vagO_in[mt * 128:(mt + 1) * 128,
                                    half * 256:(half + 1) * 256]
                            .rearrange("p (h c) -> p h c", c=64),
                            vaug[:, mt, hb + 1:hb + 8:2, 64:128])

                def q_half(half):
                    wqh = wp.tile([128, KT, 512], BF16, tag="wproj", name=_nm("wq"))
                    nc.sync.dma_start(
                        wqh[:], wq[l, :, half * 512:(half + 1) * 512]
                        .rearrange("(t p) m -> p t m", p=128))
                    grps = dense_groups()
                    for m in range(4):
                        mg = half * 4 + m
                        pt = grps[m]
                        for kk in range(KT):
                            nc.tensor.matmul(
                                pt[:], wqh[:, kk, m * 128:(m + 1) * 128],
                                x_curb[:, kk, :],
                                start=(kk == 0), stop=(kk == KT - 1))
                        nc.scalar.activation(qt[:, mg, :], pt[:], AF.Identity,
                                             bias=bq_sb[:, mg:mg + 1])

                # V before K/Q so both V collectives complete well before the
                # first attnV needs the gathered pair-V.
                v_half(0)
                k_half(0)
                v_half(1)
                nc.gpsimd.collective_compute(
                    "AllGather", OP.bypass,
                    ins=[vagE_in[:]], outs=[vagE_out[:]],
                    replica_groups=pair_groups,
                )
                nc.gpsimd.collective_compute(
                    "AllGather", OP.bypass,
                    ins=[vagO_in[:]], outs=[vagO_out[:]],
                    replica_groups=pair_groups,
                )
                k_half(1)
                q_half(0)
                q_half(1)

                # --- gathered pair K rows -> kto ---
                kto = p1.tile([128, KT, TOK], BF16, tag="kto", name=_nm("kto"))
                for g in range(KT):
                    kidx = bp.tile([128, 1], I32, tag="koidx", name=_nm("koidx"))
                    nc.sync.dma_start(
                        kidx[:], koidx[(g % 4) * 128:(g % 4 + 1) * 128, :])
                    nc.gpsimd.indirect_dma_start(
                        out=kto[:, g, :], out_offset=None,
                        in_=kag_out[g // 4][:],
                        in_offset=bass.IndirectOffsetOnAxis(ap=kidx[:, :1], axis=0),
                    )
                # --- gathered pair V -> vaug kt 4..7 (pre-masked by sender) ---
                for mt in range(4):
                    vidx = bp.tile([128, 1], I32, tag="voidx", name=_nm("voidx"))
                    nc.sync.dma_start(vidx[:], voidx[mt * 128:(mt + 1) * 128, :])
                    vstgE = p2.tile([128, 512], BF16, tag="rec", bufs=1,
                                    name=_nm("vstgE"))
                    nc.gpsimd.indirect_dma_start(
                        out=vstgE[:], out_offset=None, in_=vagE_out[:],
                        in_offset=bass.IndirectOffsetOnAxis(ap=vidx[:, :1], axis=0),
                    )
                    nc.vector.tensor_copy(
                        vaug[:, 4 + mt, 0:H:2, 0:64],
                        vstgE.rearrange("p (h c) -> p h c", c=64))
                    vstgO = p2.tile([128, 512], BF16, tag="recn", bufs=1,
                                    name=_nm("vstgO"))
                    nc.gpsimd.indirect_dma_start(
                        out=vstgO[:], out_offset=None, in_=vagO_out[:],
                        in_offset=bass.IndirectOffsetOnAxis(ap=vidx[:, :1], axis=0),
                    )
                    nc.vector.tensor_copy(
                        vaug[:, 4 + mt, 1:H:2, 64:128],
                        vstgO.rearrange("p (h c) -> p h c", c=64))

                # --- attention: software-pipelined head pairs ---
                attn = p1.tile([128, KT, TOK], BF16, tag="attn", name=_nm("attn"))

                def kt_lhs(kt, mj, prow):
                    if kt < 4:
                        return ktl[prow:prow + 64, mj, kt * 128:(kt + 1) * 128]
                    return kto[prow:prow + 64, mj, (kt - 4) * 128:(kt - 3) * 128]

                def issue_scores(mj):
                    expsE = p2.tile([128, KT, TOK], BF16, tag="exps",
                                    name=_nm("expsE"))
                    expsO = p2.tile([128, KT, TOK], BF16, tag="exps",
                                    name=_nm("expsO"))
                    for c0, c1 in ((0, 3), (3, 6), (6, 8)):
                        cs = c1 - c0
                        chE = psA()
                        chO = psA()
                        for j in range(cs):
                            kt = c0 + j
                            nc.tensor.matmul(
                                chE[:, j, :], kt_lhs(kt, mj, 0),
                                qt[0:64, mj, :], start=True, stop=True)
                            nc.tensor.matmul(
                                chO[:, j, :], kt_lhs(kt, mj, 64),
                                qt[64:128, mj, :], start=True, stop=True)
                        nc.scalar.activation(
                            expsE[:, c0:c1, :], chE[:, 0:cs, :], AF.Exp,
                            scale=DK ** -0.5)
                        nc.scalar.activation(
                            expsO[:, c0:c1, :], chO[:, 0:cs, :], AF.Exp,
                            scale=DK ** -0.5)
                    return expsE, expsO

                def issue_attnv(mj, expsE, expsO):
                    hE, hO = 2 * mj, 2 * mj + 1
                    pavE = psB()
                    pavO = psB()
                    for kt in range(KT):
                        nc.tensor.matmul(
                            pavE[:], vaug[:, kt, hE, :], expsE[:, kt, :],
                            start=(kt == 0), stop=(kt == KT - 1))
                    for kt in range(KT):
                        nc.tensor.matmul(
                            pavO[:], vaug[:, kt, hO, :], expsO[:, kt, :],
                            start=(kt == 0), stop=(kt == KT - 1))
                    # even head: out rows 0:64, denom rows 64:128; odd head
                    # mirrored.  Stack both denominator blocks into one tile,
                    # reciprocal once, DMA-swap the halves, multiply.
                    rec = p2.tile([128, TOK], F32, tag="rec", bufs=1, name=_nm("rec"))
                    recn = p2.tile([128, TOK], F32, tag="recn", bufs=1, name=_nm("recn"))
                    rec2 = p2.tile([128, TOK], F32, tag="rec2", name=_nm("rec2"))
                    num = p2.tile([128, TOK], F32, tag="num", name=_nm("num"))
                    nc.vector.tensor_copy(rec[64:128, :], pavE[64:128, :])
                    nc.vector.tensor_copy(num[0:64, :], pavE[0:64, :])
                    nc.vector.tensor_copy(rec[0:64, :], pavO[0:64, :])
                    nc.vector.tensor_copy(num[64:128, :], pavO[64:128, :])
                    nc.vector.reciprocal(recn[:], rec[:])
                    nc.sync.dma_start(rec2[0:64, :], recn[64:128, :])
                    nc.sync.dma_start(rec2[64:128, :], recn[0:64, :])
                    nc.vector.tensor_mul(attn[0:64, mj, :], num[0:64, :],
                                         rec2[0:64, :])
                    nc.vector.tensor_mul(attn[64:128, mj, :], num[64:128, :],
                                         rec2[64:128, :])

                prev = None
                for mj in range(H // 2):
                    cur = issue_scores(mj)
                    if prev is not None:
                        issue_attnv(mj - 1, *prev)
                    prev = cur
                issue_attnv(H // 2 - 1, *prev)

                # --- O-proj, residual, LN1 ---
                bo_sb = load_bias8(boT, l)
                r1 = p2.tile([128, KT, TOK], F32, tag="x", bufs=1, name=_nm("r1"))
                for half in range(2):
                    woh = wp.tile([128, KT, 512], BF16, tag="wproj", name=_nm("wo"))
                    nc.sync.dma_start(
                        woh[:], wo[l, :, half * 512:(half + 1) * 512]
                        .rearrange("(t p) m -> p t m", p=128))
                    grps = dense_groups()
                    for m in range(4):
                        mg = half * 4 + m
                        pt = grps[m]
                        for kk in range(KT):
                            nc.tensor.matmul(
                                pt[:], woh[:, kk, m * 128:(m + 1) * 128],
                                attn[:, kk, :],
                                start=(kk == 0), stop=(kk == KT - 1))
                        nc.vector.scalar_tensor_tensor(
                            out=r1[:, mg, :], in0=pt[:],
                            scalar=bo_sb[:, mg:mg + 1], in1=x_curb[:, mg, :],
                            op0=OP.add, op1=OP.add)
                g1_sb = load_bias8(g1T, l)
                be1_sb = load_bias8(be1T, l)
                x1b = layer_norm(r1, g1_sb, be1_sb)

                # --- FFN ---
                b1_sb = bp.tile([128, FT], F32, tag="bias32", name=_nm("bias32"))
                nc.sync.dma_start(b1_sb[:], b1T[l])
                b2_sb = load_bias8(b2T, l)
                r2 = p2.tile([128, KT, TOK], F32, tag="x", bufs=1, name=_nm("r2"))
                ht = p1.tile([128, FT, TOK], BF16, tag="big", name=_nm("ht"))
                for e in range(8):   # w1 eighths: dff cols e*512..
                    w1e = wp.tile([128, KT, 512], BF16, tag="wproj", name=_nm("w1e"))
                    nc.sync.dma_start(
                        w1e[:], w1[l, :, e * 512:(e + 1) * 512]
                        .rearrange("(t p) m -> p t m", p=128))
                    grps = dense_groups()
                    for m in range(4):
                        fm = e * 4 + m
                        pt = grps[m]
                        for kk in range(KT):
                            nc.tensor.matmul(
                                pt[:], w1e[:, kk, m * 128:(m + 1) * 128],
                                x1b[:, kk, :],
                                start=(kk == 0), stop=(kk == KT - 1))
                        if fm % 2 == 0:
                            nc.scalar.activation(
                                ht[:, fm, :], pt[:], AF.Relu,
                                bias=b1_sb[:, fm:fm + 1])
                        else:
                            nc.vector.tensor_scalar(
                                ht[:, fm, :], pt[:], b1_sb[:, fm:fm + 1], 0.0,
                                OP.add, OP.max)
                fps = dense_groups() + dense_groups()
                for kk in range(FT):
                    w2c = w2p.tile([128, D], BF16, tag="w2c", name=_nm("w2c"))
                    nc.sync.dma_start(w2c[:], w2[l, kk * 128:(kk + 1) * 128, :])
                    for m in range(KT):
                        nc.tensor.matmul(
                            fps[m][:], w2c[:, m * 128:(m + 1) * 128],
                            ht[:, kk, :],
                            start=(kk == 0), stop=(kk == FT - 1))
                for m in range(KT):
                    nc.vector.scalar_tensor_tensor(
                        out=r2[:, m, :], in0=fps[m][:],
                        scalar=b2_sb[:, m:m + 1],
                        in1=x1b[:, m, :],
                        op0=OP.add, op1=OP.add)

                g2_sb = load_bias8(g2T, l)
                be2_sb = load_bias8(be2T, l)
                if l == L - 1:
                    xo_final = layer_norm(r2, g2_sb, be2_sb, want_f32=True)
                else:
                    x_curb = layer_norm(r2, g2_sb, be2_sb)

            nc.sync.dma_start(
                xout.rearrange("(t p) n -> p t n", p=128), xo_final[:])

    return nc


MAXW = 1


def split_wait_overflow(nc, maxw=MAXW):
    """walrus in this toolchain rejects instructions with more than one sem
    wait; split excess waits onto preceding NoOp carriers on the same engine."""
    for f in nc.m.functions:
        for bb in f.blocks:
            if not any(i.sync_info and len(i.sync_info.on_wait) > maxw
                       for i in bb.instructions):
                continue
            newlist = []
            for inst in bb.instructions:
                si = inst.sync_info
                if si and len(si.on_wait) > maxw:
                    waits = list(si.on_wait)
                    extra, keep = waits[:-maxw], waits[-maxw:]
                    for i in range(0, len(extra), maxw):
                        newlist.append(mybir.InstNoOp(
                            name=f"{inst.name}-ws{i}", opcode="NoOp",
                            engine=inst.engine, debug=inst.debug, ins=[], outs=[],
                            sync_info=mybir.SyncInfo(
                                on_wait=extra[i:i + maxw], on_update=[]),
                        ))
                    inst.sync_info = mybir.SyncInfo(
                        on_wait=keep, on_update=list(si.on_update))
                newlist.append(inst)
            bb.instructions = newlist


def _get_nc():
    global _NC
    if _NC is None:
        _NC = _build_nc()
        split_wait_overflow(_NC)
    return _NC


def _to_bf16(a):
    return np.asarray(a, dtype=np.float32).astype(ml_dtypes.bfloat16)


def _bias_t(v, kt=KT):
    # [L, d] -> [L, 128, d//128] with column t = v[:, 128t:128t+128]
    v = np.asarray(v, dtype=np.float32)
    return np.ascontiguousarray(v.reshape(L, kt, 128).transpose(0, 2, 1))


def kernel(**inputs):
    nc = _get_nc()

    src = np.asarray(inputs["src"]).astype(np.int32).reshape(-1)      # [4096]
    src_mask = np.asarray(inputs["src_mask"]).astype(np.float32)      # [B,1,1,S]
    emb = np.asarray(inputs["emb"], dtype=np.float32)
    pe = np.asarray(inputs["pe"], dtype=np.float32)
    shared = {
        "emb": emb,
        "wq": _to_bf16(inputs["wq"]), "wk": _to_bf16(inputs["wk"]),
        "wv": _to_bf16(inputs["wv"]), "wo": _to_bf16(inputs["wo"]),
        "w1": _to_bf16(inputs["w1"]), "w2": _to_bf16(inputs["w2"]),
        "bqT": _bias_t(inputs["bq"]), "bkT": _bias_t(inputs["bk"]),
        "b1T": _bias_t(inputs["b1"], FT), "b2T": _bias_t(inputs["b2"]),
        "g1T": _bias_t(inputs["g1"]), "be1T": _bias_t(inputs["be1"]),
        "g2T": _bias_t(inputs["g2"]), "be2T": _bias_t(inputs["be2"]),
    }
    # fold the V bias through the O projection: attn rows sum to 1, so
    # out = attn@(V + bv) @ wo + bo = attn@V@wo + (bv@wo + bo)
    wo_f = np.asarray(inputs["wo"], dtype=np.float32)
    bv_f = np.asarray(inputs["bv"], dtype=np.float32)
    bo_f = np.asarray(inputs["bo"], dtype=np.float32)
    bo_eff = np.stack([bo_f[l] + bv_f[l] @ wo_f[l] for l in range(L)])
    shared["boT"] = _bias_t(bo_eff)

    in_maps = []
    for c in range(NCORES):
        b = c // 2
        half = c % 2
        m = dict(shared)
        m["src"] = np.ascontiguousarray(
            src[c * TOK:(c + 1) * TOK].reshape(TOK, 1))
        m["peT"] = np.ascontiguousarray(
            pe[half * TOK:half * TOK + TOK, :D].T.astype(np.float32))
        mb = src_mask[b, 0, 0, :]            # nonzero = keep, 0 = masked
        own = slice(half * TOK, half * TOK + TOK)
        pair = slice((1 - half) * TOK, (1 - half) * TOK + TOK)
        mb_perm = np.concatenate([mb[own], mb[pair]])
        m01 = (mb_perm != 0).astype(np.float32).reshape(KT, 128).T
        m["mask01"] = np.ascontiguousarray(m01)
        m["maskrow"] = np.ascontiguousarray(
            np.repeat(m01[:, :, None], 64, axis=2).astype(ml_dtypes.bfloat16))
        o = 1 - half  # pair-local rank of the partner
        m["koidx"] = np.ascontiguousarray(
            (np.arange(TOK, dtype=np.int32) + o * TOK).reshape(TOK, 1))
        m["voidx"] = np.ascontiguousarray(
            (np.arange(TOK, dtype=np.int32) + o * TOK).reshape(TOK, 1))
        in_maps.append(m)

    res = run_bass_kernel_spmd(nc, in_maps, list(range(NCORES)))
    out = np.empty((B * S, D), dtype=np.float32)
    for c in range(NCORES):
        out[c * TOK:(c + 1) * TOK] = res.results[c]["xout"].T
    return out.reshape(B, S, D)
